# revision 3
# baseline (speedup 1.0000x reference)
"""Trainium2 Bass kernel for GrowableTwistorLNN (T=512, B=64, D=256, H=1024, O=64).

Strategy: data-parallel over batch across 8 NeuronCores (per sharding hint).
Each core runs the full sequential recurrence for its 8-sample batch slice:
  - state z (real/imag) kept transposed as [128 part = h%128, 64 free = (h//128, b)]
  - per step: 3 HxH matmuls in bf16 (stationary = constant weight tiles
    W.T[k,m], moving = tanh/|z| tiles, fp32 PSUM accumulation over 8 k-tiles)
  - elementwise tau/clip/update chain on DVE/ACT in fp32
  - fp32 readout matmul per step
  - U(x) drive precomputed on-device (x.T @ U_w.T) into SBUF in bf16
Recurrence runs in a tc.For_i dynamic loop (instruction count stays small).
"""
import numpy as np
import ml_dtypes

import concourse.bass as bass
import concourse.bacc as bacc
import concourse.mybir as mybir
import concourse.tile as tile
from concourse import bass_utils

T, B, D, H, O = 512, 64, 256, 1024, 64
NC_ = 8
BL = B // NC_          # batch per core = 8
M = H // 128           # 8 h'-tiles
KD = D // 128          # 2 d-tiles
f32 = mybir.dt.float32
bf16 = mybir.dt.bfloat16
bfnp = ml_dtypes.bfloat16

_cache = {}


def _build(t_steps=T):
    nc = bacc.Bacc("TRN2", target_bir_lowering=False, debug=False, num_devices=NC_)
    xT_d = nc.dram_tensor("xT", [D, t_steps * BL], bf16, kind="ExternalInput")
    wr_d = nc.dram_tensor("wrT", [128, M * M * 128], bf16, kind="ExternalInput")
    wi_d = nc.dram_tensor("wiT", [128, M * M * 128], bf16, kind="ExternalInput")
    wt_d = nc.dram_tensor("wtT", [128, M * M * 128], bf16, kind="ExternalInput")
    uw_d = nc.dram_tensor("uwT", [128, M * KD * 128], bf16, kind="ExternalInput")
    ow_d = nc.dram_tensor("owT", [128, M * O], f32, kind="ExternalInput")
    br_d = nc.dram_tensor("biasr", [128, M], f32, kind="ExternalInput")
    wtb_d = nc.dram_tensor("wtbf", [128, M * BL], f32, kind="ExternalInput")
    tb_d = nc.dram_tensor("taubf", [128, M * BL], f32, kind="ExternalInput")
    bd_d = nc.dram_tensor("bdifff", [128, M * BL], f32, kind="ExternalInput")
    y_d = nc.dram_tensor("y", [t_steps + 1, O, BL], f32, kind="ExternalOutput")

    FB = M * BL  # 64 free columns of a state tile

    with tile.TileContext(nc, trace_sim=False) as tc:
        with (
            tc.tile_pool(name="const", bufs=1) as cpool,
            tc.tile_pool(name="ux", bufs=1) as uxpool,
            tc.tile_pool(name="state", bufs=1) as spool,
            tc.tile_pool(name="work", bufs=2) as wpool,
            tc.tile_pool(name="xin", bufs=3) as xpool,
            tc.tile_pool(name="psum", bufs=1, space="PSUM") as ppool,
            tc.tile_pool(name="psux", bufs=2, space="PSUM") as puxpool,
        ):
            wr_sb = cpool.tile([128, M * M * 128], bf16)
            wi_sb = cpool.tile([128, M * M * 128], bf16)
            wt_sb = cpool.tile([128, M * M * 128], bf16)
            uw_sb = cpool.tile([128, M * KD * 128], bf16)
            ow_sb = cpool.tile([128, M * O], f32)
            br_sb = cpool.tile([128, M], f32)
            wtb_sb = cpool.tile([128, FB], f32)
            tb_sb = cpool.tile([128, FB], f32)
            bd_sb = cpool.tile([128, FB], f32)
            uxr_sb = uxpool.tile([128, t_steps * FB], bf16)

            for sb, d in [(wr_sb, wr_d), (wi_sb, wi_d), (wt_sb, wt_d),
                          (uw_sb, uw_d), (ow_sb, ow_d), (br_sb, br_d),
                          (wtb_sb, wtb_d), (tb_sb, tb_d), (bd_sb, bd_d)]:
                nc.sync.dma_start(sb[:], d.ap())

            wr_v = wr_sb[:].rearrange("p (m k c) -> p m k c", m=M, k=M)
            wi_v = wi_sb[:].rearrange("p (m k c) -> p m k c", m=M, k=M)
            wt_v = wt_sb[:].rearrange("p (m k c) -> p m k c", m=M, k=M)
            uw_v = uw_sb[:].rearrange("p (m k c) -> p m k c", m=M, k=KD)
            ow_v = ow_sb[:].rearrange("p (k o) -> p k o", k=M)
            ux_v = uxr_sb[:].rearrange("p (t m b) -> p t m b", t=t_steps, m=M)

            zr = spool.tile([128, FB], f32)
            zi = spool.tile([128, FB], f32)
            g2r = spool.tile([128, FB], bf16)
            g2i = spool.tile([128, FB], bf16)
            g2m = spool.tile([128, FB], bf16)
            nc.vector.memset(zr[:], 0.0)
            nc.vector.memset(zi[:], 0.0)
            nc.vector.memset(g2r[:], 0.0)
            nc.vector.memset(g2i[:], 0.0)
            nc.vector.memset(g2m[:], 1e-10)
            eps_sb = spool.tile([128, 1], f32)
            nc.vector.memset(eps_sb[:], 1e-20)

            # ---- U(x) phase: uxr[:, t, m, b] = (x @ U_w.T + U_b + b_real).T ----
            n_chunk = t_steps * BL // 512  # 512 (t,b) pairs per chunk
            tpc = 512 // BL                # t's per chunk
            for ch in range(n_chunk):
                xk0 = xpool.tile([128, 512], bf16, tag="xk")
                xk1 = xpool.tile([128, 512], bf16, tag="xk")
                nc.sync.dma_start(xk0[:], xT_d.ap()[0:128, ch * 512:(ch + 1) * 512])
                nc.sync.dma_start(xk1[:], xT_d.ap()[128:256, ch * 512:(ch + 1) * 512])
                for m in range(M):
                    pux = puxpool.tile([128, 512], f32, tag="pux")
                    nc.tensor.matmul(pux[:], uw_v[:, m, 0, :], xk0[:], start=True, stop=False)
                    nc.tensor.matmul(pux[:], uw_v[:, m, 1, :], xk1[:], start=False, stop=True)
                    out = ux_v[:, ch * tpc:(ch + 1) * tpc, m, :]
                    nc.scalar.activation(out, pux[:].rearrange("p (t b) -> p t b", b=BL),
                                         mybir.ActivationFunctionType.Identity,
                                         bias=br_sb[:, m:m + 1])

            # ---- recurrence ----
            def body(iv):
                # readout of PREVIOUS step's state (z_iv) -> y_d[iv]; host drops row 0
                py = ppool.tile([O, BL], f32, tag="py")
                for k in range(M):
                    nc.tensor.matmul(py[:], ow_v[:, k, :], zr[:, k * BL:(k + 1) * BL],
                                     start=(k == 0), stop=(k == M - 1))
                yb = wpool.tile([O, BL], f32, tag="yb")
                nc.scalar.copy(yb[:], py[:])
                nc.sync.dma_start(y_d.ap()[bass.ds(iv, 1), :, :], yb[:])

                pr = ppool.tile([128, FB], f32, tag="pr")
                pi = ppool.tile([128, FB], f32, tag="pi")
                pt = ppool.tile([128, FB], f32, tag="pt")
                for m in range(M):
                    for k in range(M):
                        st, sp = (k == 0), (k == M - 1)
                        nc.tensor.matmul(pr[:, m * BL:(m + 1) * BL], wr_v[:, m, k, :],
                                         g2r[:, k * BL:(k + 1) * BL], start=st, stop=sp)
                for m in range(M):
                    for k in range(M):
                        st, sp = (k == 0), (k == M - 1)
                        nc.tensor.matmul(pi[:, m * BL:(m + 1) * BL], wi_v[:, m, k, :],
                                         g2i[:, k * BL:(k + 1) * BL], start=st, stop=sp)
                for m in range(M):
                    for k in range(M):
                        st, sp = (k == 0), (k == M - 1)
                        nc.tensor.matmul(pt[:, m * BL:(m + 1) * BL], wt_v[:, m, k, :],
                                         g2m[:, k * BL:(k + 1) * BL], start=st, stop=sp)

                AL = mybir.AluOpType
                uxcol = uxr_sb[:, bass.ts(iv, FB)]
                # tau chain
                tw = wpool.tile([128, FB], f32, tag="tw")
                nc.vector.tensor_tensor(tw[:], pt[:], wtb_sb[:], AL.add)
                sg = wpool.tile([128, FB], f32, tag="sg")
                nc.scalar.activation(sg[:], tw[:], mybir.ActivationFunctionType.Sigmoid)
                nc.vector.tensor_tensor(sg[:], sg[:], tb_sb[:], AL.add)
                nc.vector.tensor_scalar(sg[:], sg[:], 1.0, 0.01, AL.min, AL.max)
                nc.vector.tensor_scalar(sg[:], sg[:], 10.0, 1e-5, AL.mult, AL.add)
                r01 = wpool.tile([128, FB], f32, tag="r01")
                nc.vector.reciprocal(r01[:], sg[:])
                # real path
                dr = wpool.tile([128, FB], f32, tag="dr")
                nc.vector.tensor_tensor(dr[:], pr[:], zr[:], AL.subtract)
                nc.vector.tensor_tensor(dr[:], dr[:], uxcol, AL.add)
                nc.vector.tensor_tensor(dr[:], dr[:], r01[:], AL.mult)
                nc.vector.tensor_scalar(dr[:], dr[:], 1.0, -1.0, AL.min, AL.max)
                nc.vector.tensor_tensor(zr[:], zr[:], dr[:], AL.add)
                nc.vector.tensor_scalar(zr[:], zr[:], 100.0, -100.0, AL.min, AL.max)
                # imag path
                di = wpool.tile([128, FB], f32, tag="di")
                nc.vector.tensor_tensor(di[:], pi[:], zi[:], AL.subtract)
                nc.vector.tensor_tensor(di[:], di[:], uxcol, AL.add)
                nc.vector.tensor_tensor(di[:], di[:], bd_sb[:], AL.add)
                nc.vector.tensor_tensor(di[:], di[:], r01[:], AL.mult)
                nc.vector.tensor_scalar(di[:], di[:], 1.0, -1.0, AL.min, AL.max)
                nc.vector.tensor_tensor(zi[:], zi[:], di[:], AL.add)
                nc.vector.tensor_scalar(zi[:], zi[:], 100.0, -100.0, AL.min, AL.max)
                # next step's matmul operands
                s = wpool.tile([128, FB], f32, tag="s")
                nc.vector.tensor_tensor(s[:], zr[:], zr[:], AL.mult)
                s2 = wpool.tile([128, FB], f32, tag="s2")
                nc.vector.tensor_tensor(s2[:], zi[:], zi[:], AL.mult)
                nc.vector.tensor_tensor(s[:], s[:], s2[:], AL.add)
                nc.scalar.activation(g2r[:], zr[:], mybir.ActivationFunctionType.Tanh)
                nc.scalar.activation(g2i[:], zi[:], mybir.ActivationFunctionType.Tanh)
                nc.scalar.activation(g2m[:], s[:], mybir.ActivationFunctionType.Sqrt,
                                     bias=eps_sb[:, 0:1])

            with tc.For_i(0, t_steps, 1, hint_engines=(mybir.EngineType.PE,)) as iv:
                body(iv)

            # epilogue: readout of final state -> y_d[t_steps]
            py = ppool.tile([O, BL], f32, tag="py")
            for k in range(M):
                nc.tensor.matmul(py[:], ow_v[:, k, :], zr[:, k * BL:(k + 1) * BL],
                                 start=(k == 0), stop=(k == M - 1))
            yb = wpool.tile([O, BL], f32, tag="yb")
            nc.scalar.copy(yb[:], py[:])
            nc.sync.dma_start(y_d.ap()[t_steps:t_steps + 1, :, :], yb[:])

    nc.compile()
    return nc


def _prep_inputs(x, W_real, W_imag, U_w, U_b, W_tau_w, W_tau_b,
                 mask_real, mask_imag, tau_bias, b_real, b_imag, out_w, out_b,
                 t_steps=T):
    def sig(v):
        return 1.0 / (1.0 + np.exp(-v))

    Wr = (W_real * sig(mask_real)).astype(np.float32)
    Wi = (W_imag * sig(mask_imag)).astype(np.float32)
    Wt = np.asarray(W_tau_w, np.float32)

    def wpack(Wm):  # [p, m, k, c] -> [128, M*M*128]  lhsT[m,k][p,c] = W[m*128+c, k*128+p]
        A = Wm.reshape(M, 128, M, 128)          # [m, c, k, p]
        return np.ascontiguousarray(A.transpose(3, 0, 2, 1).reshape(128, M * M * 128)).astype(bfnp)

    Uw = np.asarray(U_w, np.float32).reshape(M, 128, KD, 128)   # [m, c, kd, p]
    uwp = np.ascontiguousarray(Uw.transpose(3, 0, 2, 1).reshape(128, M * KD * 128)).astype(bfnp)
    Ow = np.asarray(out_w, np.float32).reshape(O, M, 128)       # [o, k, p]
    owp = np.ascontiguousarray(Ow.transpose(2, 1, 0).reshape(128, M * O)).astype(np.float32)

    biasr = (np.asarray(U_b) + np.asarray(b_real)).astype(np.float32).reshape(M, 128).T
    biasr = np.ascontiguousarray(biasr)

    def full_h(vec):  # [H] -> [128, M*BL] at [p, m*BL+b] = vec[m*128+p]
        v = np.asarray(vec, np.float32).reshape(M, 128).T       # [p, m]
        return np.ascontiguousarray(np.repeat(v[:, :, None], BL, axis=2).reshape(128, M * BL))

    wtbf = full_h(W_tau_b)
    taubf = full_h(tau_bias)
    bdifff = full_h(np.asarray(b_imag) - np.asarray(b_real))

    x = np.asarray(x, np.float32)
    in_maps = []
    for c in range(NC_):
        xc = x[:t_steps, c * BL:(c + 1) * BL, :].reshape(t_steps * BL, D)
        xT = np.ascontiguousarray(xc.T).astype(bfnp)
        in_maps.append({
            "xT": xT, "wrT": wpack(Wr), "wiT": wpack(Wi), "wtT": wpack(Wt),
            "uwT": uwp, "owT": owp, "biasr": biasr,
            "wtbf": wtbf, "taubf": taubf, "bdifff": bdifff,
        })
    return in_maps


def kernel(**inputs):
    t_steps = T
    if "nc" not in _cache:
        _cache["nc"] = _build(t_steps)
    nc = _cache["nc"]
    in_maps = _prep_inputs(**inputs, t_steps=t_steps)
    res = bass_utils.run_bass_kernel_spmd(nc, in_maps, core_ids=list(range(NC_)))
    out_b = np.asarray(inputs["out_b"], np.float32)
    y = np.zeros((t_steps, B, O), np.float32)
    for c in range(NC_):
        yc = res.results[c]["y"][1:]                      # [T, O, BL]
        y[:, c * BL:(c + 1) * BL, :] = yc.transpose(0, 2, 1)
    return y + out_b
